# revision 1
# baseline (speedup 1.0000x reference)
"""ConvSTFT on Trainium2: strided conv of x[32, 480000] against a fixed
[514, 1, 400] Fourier basis, hop 100 -> out [32, 514, 4803] f32.

Sharding: pure data parallel. Batch dim (32) split 4-per-core across 8
NeuronCores; the small weight is replicated.

Host prep (sharding layer): pad x by 300 on both sides, then lay it out
chunk-transposed in blocks of 128 hops:
    x_dev[b, s, r, p] = x_padded[b, (128 s + p) * 100 + r]
so the device can DMA straight into XT[r, f'] = x_padded[100 f' + r]
(f' = 128 s + p) with 256-byte contiguous lines. The weight is passed
transposed: wt[t, c] = weight[c, 0, t]. Both are cast to bf16.

Per-core device kernel (Bass/Tile):
  t = 100j + r decomposition (j in 0..3, r in 0..99) turns the overlapped
  conv into 4 PSUM-accumulated matmuls:
      out[c, f] = sum_j sum_r wt[100j + r, c] * XT[r, f + j]
  - lhsT = wt[r, j, c-tile] (K=100, M<=128), rhs = XT[r, f-tile] (N<=512),
    fp32 PSUM accumulation over j, all 8 PSUM banks in flight.
  - PSUM evacuated alternately by DVE/ACT into an SBUF row [<=128, 4803]
    f32, stored with two large contiguous DMAs per (batch, channel-tile).
  - Startup: warmup matmuls open the HAM clock gate while the critical
    first loads run on the two parallel HWDGE rings (weights on ACT, first
    XT piece on SP); later loads queue FIFO behind them so they cannot
    starve the pipeline.
This streams the PE at its floor (1 bf16 column/cycle, 216 ns per N=512
matmul measured; 20 tile-streams per frame-column = ceil(514/128) *
ceil(400/128) is provably minimal). Measured ~185 us/core vs ~162 us PE
stream floor; PE occupancy ~90% with no gaps >300 ns in steady state.
"""

import numpy as np
import ml_dtypes

WIN, HOP, C = 400, 100, 514
B, T = 32, 480000
PAD = WIN - HOP                       # 300
N_CORES = 8
B_LOC = B // N_CORES                  # 4
T_PAD = T + 2 * PAD                   # 480600
N_FRAMES = (T_PAD - WIN) // HOP + 1   # 4803
S_BLOCKS = -(-(T_PAD // HOP) // 128)  # 38 blocks of 128 chunks
N_CHUNKS = S_BLOCKS * 128             # 4864
NJ = WIN // HOP                       # 4

F_TILE = 512
C_TILE = 128
LOAD_GRP = 8                          # s-blocks per input DMA piece
STORE_SPLIT = 5                       # store first half after this many ftiles


def build_program(b_loc=B_LOC, s_blocks=S_BLOCKS, n_frames=N_FRAMES):
    import concourse.bacc as bacc
    import concourse.mybir as mybir
    import concourse.tile as tile

    dt = mybir.dt
    n_chunks = s_blocks * 128
    assert n_frames + NJ - 1 <= n_chunks

    nc = bacc.Bacc("TRN2", target_bir_lowering=False, debug=False)
    x_d = nc.dram_tensor(
        "x", [b_loc, s_blocks, HOP, 128], dt.bfloat16, kind="ExternalInput"
    ).ap()
    w_d = nc.dram_tensor("wt", [WIN, C], dt.bfloat16, kind="ExternalInput").ap()
    o_d = nc.dram_tensor(
        "out", [b_loc, C, n_frames], dt.float32, kind="ExternalOutput"
    ).ap()

    ctiles = [(c0, min(C_TILE, C - c0)) for c0 in range(0, C, C_TILE)]
    ftiles = [(f0, min(F_TILE, n_frames - f0)) for f0 in range(0, n_frames, F_TILE)]

    n_ct, n_ft = len(ctiles), len(ftiles)
    mid = ftiles[STORE_SPLIT][0] if n_ft > STORE_SPLIT else 0

    with tile.TileContext(nc) as tc:
        with (
            tc.tile_pool(name="const", bufs=1) as constp,
            tc.tile_pool(name="xt", bufs=2) as xtp,
            tc.tile_pool(name="orow", bufs=7) as orowp,
            tc.tile_pool(name="mmps", bufs=8, space="PSUM") as mmps,
        ):
            # Warm the PE clock gate (HAM) with throwaway matmuls while the
            # first input DMAs are in flight (needs ~3.4us of sustained PE
            # activity to lift the clock from 1.2 to 2.4 GHz).
            warm = constp.tile([128, 512], dt.bfloat16)
            nc.gpsimd.memset(warm[:], 0.0)
            wps = mmps.tile([128, F_TILE], dt.float32, tag="ps")
            for _ in range(12):
                nc.tensor.matmul(wps[0:16, :], warm[:, 0:16], warm[:])

            # critical first loads on the two parallel HWDGE rings:
            # weights on ACT, first xt piece on SP
            wsb = constp.tile([HOP, NJ, C], dt.bfloat16)
            nc.scalar.dma_start(wsb[:], w_d.rearrange("(j r) c -> r j c", r=HOP))
            first = min(5, s_blocks)
            xt0 = xtp.tile([HOP, s_blocks, 128], dt.bfloat16, tag="xt")
            nc.sync.dma_start(
                xt0[:, 0:first, :], x_d[0, 0:first].rearrange("g r p -> r g p")
            )
            for g0 in range(first, s_blocks, 6):
                gs = min(6, s_blocks - g0)
                nc.scalar.dma_start(
                    xt0[:, g0 : g0 + gs, :],
                    x_d[0, g0 : g0 + gs].rearrange("g r p -> r g p"),
                )

            ncopy = 0

            def mm_group(xtf, orow, b, c0, cm, f0, fn):
                nonlocal ncopy
                ps = mmps.tile([128, F_TILE], dt.float32, tag="ps")
                for j in range(NJ):
                    nc.tensor.matmul(
                        ps[0:cm, 0:fn],
                        wsb[0:HOP, j, c0 : c0 + cm],
                        xtf[0:HOP, f0 + j : f0 + j + fn],
                        start=(j == 0),
                        stop=(j == NJ - 1),
                    )
                # alternate evacuation between DVE and ACT
                if ncopy % 2 == 1:
                    nc.scalar.copy(orow[0:cm, f0 : f0 + fn], ps[0:cm, 0:fn])
                else:
                    nc.vector.tensor_copy(orow[0:cm, f0 : f0 + fn], ps[0:cm, 0:fn])
                ncopy += 1

            for b in range(b_loc):
                if b == 0:
                    xt = xt0
                else:
                    # later batches queue behind b0 on the same FIFO ring,
                    # so they cannot starve the critical first loads
                    xt = xtp.tile([HOP, s_blocks, 128], dt.bfloat16, tag="xt")
                    for g0 in range(0, s_blocks, LOAD_GRP):
                        gs = min(LOAD_GRP, s_blocks - g0)
                        nc.scalar.dma_start(
                            xt[:, g0 : g0 + gs, :],
                            x_d[b, g0 : g0 + gs].rearrange("g r p -> r g p"),
                        )
                xtf = xt.rearrange("r g p -> r (g p)")

                for c0, cm in ctiles:
                    orow = orowp.tile([128, n_frames], dt.float32, tag="orow")
                    for fi, (f0, fn) in enumerate(ftiles):
                        mm_group(xtf, orow, b, c0, cm, f0, fn)
                        if fi == STORE_SPLIT - 1 and n_ft > STORE_SPLIT:
                            nc.sync.dma_start(
                                o_d[b, c0 : c0 + cm, 0:mid], orow[0:cm, 0:mid]
                            )
                    nc.sync.dma_start(
                        o_d[b, c0 : c0 + cm, mid:n_frames],
                        orow[0:cm, mid:n_frames],
                    )

    nc.compile()
    return nc


_NC = None
LAST_RESULTS = None


def _ensure_axon_hooks_stub():
    """If BASS_TRACE is set but the container's antenv lacks axon_hooks,
    run_bass_kernel_spmd would crash on import; degrade to no-trace."""
    import sys

    try:
        import antenv.axon_hooks  # noqa: F401
    except ImportError:
        import types

        import antenv

        m = types.ModuleType("antenv.axon_hooks")
        m.get_axon_ntff_profile_hook = lambda: None
        m.set_axon_ntff_profile_hook = lambda h: None
        sys.modules["antenv.axon_hooks"] = m
        antenv.axon_hooks = m


def _prep_inputs(x, weight):
    x = np.asarray(x, dtype=np.float32)
    w = np.asarray(weight, dtype=np.float32)
    nb = x.shape[0]
    xp = np.zeros((nb, N_CHUNKS * HOP), dtype=np.float32)
    xp[:, PAD : PAD + x.shape[1]] = x
    # chunk-block mini-transpose: [b, s, p, r] -> [b, s, r, p]
    xdev = np.ascontiguousarray(
        xp.reshape(nb, S_BLOCKS, 128, HOP).transpose(0, 1, 3, 2)
    ).astype(ml_dtypes.bfloat16)
    wt = np.ascontiguousarray(w.reshape(C, WIN).T).astype(ml_dtypes.bfloat16)
    return xdev, wt


def kernel(x, weight):
    global _NC, LAST_RESULTS
    from concourse.bass_utils import run_bass_kernel_spmd

    _ensure_axon_hooks_stub()
    xdev, wt = _prep_inputs(x, weight)
    if _NC is None:
        _NC = build_program()
    in_maps = [
        {"x": np.ascontiguousarray(xdev[c * B_LOC : (c + 1) * B_LOC]), "wt": wt}
        for c in range(N_CORES)
    ]
    res = run_bass_kernel_spmd(_NC, in_maps, core_ids=list(range(N_CORES)))
    LAST_RESULTS = res
    out = np.concatenate([r["out"] for r in res.results], axis=0)
    return np.ascontiguousarray(out)



# revision 2
# speedup vs baseline: 1.2204x; 1.2204x over previous
"""ConvSTFT on Trainium2: strided conv of x[32, 480000] against a fixed
[514, 1, 400] Fourier basis, hop 100 -> out [32, 514, 4803] f32.

Sharding: pure data parallel. Batch dim (32) split 4-per-core across 8
NeuronCores; the small weight is replicated.

Split of work: the PE matmul cost is (#streams) x N cycles, with
#streams = ceil(C/128) * ceil(WIN/128) per frame-column. C=514 needs 5
channel tiles, the 5th holding only 2 channels -- 25% wasted PE time.
So the device computes only channels 0..511 (4 full tiles, the bf16 PE
floor of 16 streams/frame-column), while the host computes the last 2
channels with one small BLAS GEMM over the strided frame view (<1% of
the FLOPs). Device output is stored bf16 (halves the dominant output
DMA: 39.3 -> 19.7 MB/core) and upcast on the host.

Host prep (sharding layer): pad x by 300 on both sides, then lay it out
chunk-transposed in blocks of 128 hops:
    x_dev[b, s, r, p] = x_padded[b, (128 s + p) * 100 + r]
so the device can DMA straight into XT[r, f'] = x_padded[100 f' + r]
(f' = 128 s + p) with 256-byte contiguous lines. The weight is passed
transposed: wt[t, c] = weight[c, 0, t]. Both are cast to bf16.

Per-core device kernel (Bass/Tile):
  t = 100j + r decomposition (j in 0..3, r in 0..99) turns the overlapped
  conv into 4 PSUM-accumulated matmuls:
      out[c, f] = sum_j sum_r wt[100j + r, c] * XT[r, f + j]
  - lhsT = wt[r, j, c-tile] (K=100, M=128), rhs = XT[r, f-tile] (N<=512),
    fp32 PSUM accumulation over j, all 8 PSUM banks in flight.
  - PSUM evacuated alternately by DVE/ACT into an SBUF row [128, 4803]
    bf16 (cast on copy), stored with two large contiguous DMAs per
    (batch, channel-tile).
  - Startup: warmup matmuls open the HAM clock gate while the critical
    first loads run on the two parallel HWDGE rings (weights on ACT, first
    XT piece on SP); later loads queue FIFO behind them so they cannot
    starve the pipeline.
"""

import numpy as np
import ml_dtypes

WIN, HOP, C = 400, 100, 514
C_DEV = 512                           # channels computed on device
B, T = 32, 480000
PAD = WIN - HOP                       # 300
N_CORES = 8
B_LOC = B // N_CORES                  # 4
T_PAD = T + 2 * PAD                   # 480600
N_FRAMES = (T_PAD - WIN) // HOP + 1   # 4803
S_BLOCKS = -(-(T_PAD // HOP) // 128)  # 38 blocks of 128 chunks
N_CHUNKS = S_BLOCKS * 128             # 4864
NJ = WIN // HOP                       # 4

F_TILE = 512
C_TILE = 128
LOAD_GRP = 8                          # s-blocks per input DMA piece
STORE_SPLIT = 5                       # store first half after this many ftiles


def build_program(b_loc=B_LOC, s_blocks=S_BLOCKS, n_frames=N_FRAMES):
    import concourse.bacc as bacc
    import concourse.mybir as mybir
    import concourse.tile as tile

    dt = mybir.dt
    n_chunks = s_blocks * 128
    assert n_frames + NJ - 1 <= n_chunks

    nc = bacc.Bacc("TRN2", target_bir_lowering=False, debug=False)
    x_d = nc.dram_tensor(
        "x", [b_loc, s_blocks, HOP, 128], dt.bfloat16, kind="ExternalInput"
    ).ap()
    w_d = nc.dram_tensor("wt", [WIN, C_DEV], dt.bfloat16, kind="ExternalInput").ap()
    o_d = nc.dram_tensor(
        "out", [b_loc, C_DEV, n_frames], dt.bfloat16, kind="ExternalOutput"
    ).ap()

    ctiles = [(c0, min(C_TILE, C_DEV - c0)) for c0 in range(0, C_DEV, C_TILE)]
    ftiles = [(f0, min(F_TILE, n_frames - f0)) for f0 in range(0, n_frames, F_TILE)]

    n_ct, n_ft = len(ctiles), len(ftiles)
    mid = ftiles[STORE_SPLIT][0] if n_ft > STORE_SPLIT else 0

    with tile.TileContext(nc) as tc:
        with (
            tc.tile_pool(name="const", bufs=1) as constp,
            tc.tile_pool(name="xt", bufs=2) as xtp,
            tc.tile_pool(name="orow", bufs=7) as orowp,
            tc.tile_pool(name="mmps", bufs=8, space="PSUM") as mmps,
        ):
            # Warm the PE clock gate (HAM) with throwaway matmuls while the
            # first input DMAs are in flight (needs ~3.4us of sustained PE
            # activity to lift the clock from 1.2 to 2.4 GHz).
            warm = constp.tile([128, 512], dt.bfloat16)
            nc.gpsimd.memset(warm[:], 0.0)
            wps = mmps.tile([128, F_TILE], dt.float32, tag="ps")
            for _ in range(12):
                nc.tensor.matmul(wps[0:16, :], warm[:, 0:16], warm[:])

            # critical first loads on the two parallel HWDGE rings:
            # weights on ACT, first xt piece on SP
            wsb = constp.tile([HOP, NJ, C_DEV], dt.bfloat16)
            nc.scalar.dma_start(wsb[:], w_d.rearrange("(j r) c -> r j c", r=HOP))
            first = min(5, s_blocks)
            xt0 = xtp.tile([HOP, s_blocks, 128], dt.bfloat16, tag="xt")
            nc.sync.dma_start(
                xt0[:, 0:first, :], x_d[0, 0:first].rearrange("g r p -> r g p")
            )
            for g0 in range(first, s_blocks, 6):
                gs = min(6, s_blocks - g0)
                nc.scalar.dma_start(
                    xt0[:, g0 : g0 + gs, :],
                    x_d[0, g0 : g0 + gs].rearrange("g r p -> r g p"),
                )

            ncopy = 0

            def mm_group(xtf, orow, b, c0, cm, f0, fn):
                nonlocal ncopy
                ps = mmps.tile([128, F_TILE], dt.float32, tag="ps")
                for j in range(NJ):
                    nc.tensor.matmul(
                        ps[0:cm, 0:fn],
                        wsb[0:HOP, j, c0 : c0 + cm],
                        xtf[0:HOP, f0 + j : f0 + j + fn],
                        start=(j == 0),
                        stop=(j == NJ - 1),
                    )
                # alternate evacuation between DVE and ACT (casts f32->bf16)
                if ncopy % 2 == 1:
                    nc.scalar.copy(orow[0:cm, f0 : f0 + fn], ps[0:cm, 0:fn])
                else:
                    nc.vector.tensor_copy(orow[0:cm, f0 : f0 + fn], ps[0:cm, 0:fn])
                ncopy += 1

            for b in range(b_loc):
                if b == 0:
                    xt = xt0
                else:
                    # later batches queue behind b0 on the same FIFO ring,
                    # so they cannot starve the critical first loads
                    xt = xtp.tile([HOP, s_blocks, 128], dt.bfloat16, tag="xt")
                    for g0 in range(0, s_blocks, LOAD_GRP):
                        gs = min(LOAD_GRP, s_blocks - g0)
                        nc.scalar.dma_start(
                            xt[:, g0 : g0 + gs, :],
                            x_d[b, g0 : g0 + gs].rearrange("g r p -> r g p"),
                        )
                xtf = xt.rearrange("r g p -> r (g p)")

                for c0, cm in ctiles:
                    orow = orowp.tile([128, n_frames], dt.bfloat16, tag="orow")
                    for fi, (f0, fn) in enumerate(ftiles):
                        mm_group(xtf, orow, b, c0, cm, f0, fn)
                        if fi == STORE_SPLIT - 1 and n_ft > STORE_SPLIT:
                            nc.sync.dma_start(
                                o_d[b, c0 : c0 + cm, 0:mid], orow[0:cm, 0:mid]
                            )
                    nc.sync.dma_start(
                        o_d[b, c0 : c0 + cm, mid:n_frames],
                        orow[0:cm, mid:n_frames],
                    )

    nc.compile()
    return nc


_NC = None
LAST_RESULTS = None


def _ensure_axon_hooks_stub():
    """If BASS_TRACE is set but the container's antenv lacks axon_hooks,
    run_bass_kernel_spmd would crash on import; degrade to no-trace."""
    import sys

    try:
        import antenv.axon_hooks  # noqa: F401
    except ImportError:
        import types

        import antenv

        m = types.ModuleType("antenv.axon_hooks")
        m.get_axon_ntff_profile_hook = lambda: None
        m.set_axon_ntff_profile_hook = lambda h: None
        sys.modules["antenv.axon_hooks"] = m
        antenv.axon_hooks = m


def _prep_inputs(x, weight):
    x = np.asarray(x, dtype=np.float32)
    w = np.asarray(weight, dtype=np.float32)
    nb = x.shape[0]
    xp = np.zeros((nb, N_CHUNKS * HOP), dtype=np.float32)
    xp[:, PAD : PAD + x.shape[1]] = x
    # chunk-block mini-transpose: [b, s, p, r] -> [b, s, r, p]
    xdev = np.ascontiguousarray(
        xp.reshape(nb, S_BLOCKS, 128, HOP).transpose(0, 1, 3, 2)
    ).astype(ml_dtypes.bfloat16)
    wt = np.ascontiguousarray(w.reshape(C, WIN)[:C_DEV].T).astype(ml_dtypes.bfloat16)
    return xp, xdev, wt


def _host_tail_channels(xp, w):
    """Channels C_DEV..C-1 via one BLAS GEMM over the strided frame view."""
    w2 = np.ascontiguousarray(
        np.asarray(w, dtype=np.float32).reshape(C, WIN)[C_DEV:].T
    )  # [WIN, C - C_DEV]
    v = np.lib.stride_tricks.sliding_window_view(xp, WIN, axis=1)[:, ::HOP, :]
    v = v[:, :N_FRAMES]  # [B, N_FRAMES, WIN]
    out2 = np.tensordot(v, w2, axes=([2], [0]))  # [B, N_FRAMES, C-C_DEV]
    return np.ascontiguousarray(out2.transpose(0, 2, 1))


def kernel(x, weight):
    global _NC, LAST_RESULTS
    from concourse.bass_utils import run_bass_kernel_spmd

    _ensure_axon_hooks_stub()
    xp, xdev, wt = _prep_inputs(x, weight)
    tail = _host_tail_channels(xp, weight)
    if _NC is None:
        _NC = build_program()
    in_maps = [
        {"x": np.ascontiguousarray(xdev[c * B_LOC : (c + 1) * B_LOC]), "wt": wt}
        for c in range(N_CORES)
    ]
    res = run_bass_kernel_spmd(_NC, in_maps, core_ids=list(range(N_CORES)))
    LAST_RESULTS = res
    out = np.empty((B, C, N_FRAMES), dtype=np.float32)
    for c in range(N_CORES):
        out[c * B_LOC : (c + 1) * B_LOC, :C_DEV] = res.results[c]["out"]
    out[:, C_DEV:] = tail
    return out
